# revision 3
# baseline (speedup 1.0000x reference)
"""Trainium2 Bass kernel for nn_MultiHeadAttention_56676388438432 (v2).

Reference math (all H=16 heads share identical weights, so they collapse):
    q = query @ Wq; k = key @ Wk; v = value @ Wv          (full-width, [B,S,D])
    qh = q @ wq_h + bq_h                                   ([B,S,64])
    scores = qh @ kh^T / 8, causal mask, softmax
    out_h = attn @ vh
    out = tile(out_h, 16) @ Wo

Algebraic collapse (exact):
    qh = query @ (Wq @ wq_h / 8) + bq_h/8   -> combined Aq [1024, 64]
    kh = key   @ (Wk @ wk_h) + bk_h         -> combined Ak
    vh = value @ (Wv @ wv_h)        (bv_h folded into Wo, see below)
    out = [out_h ; expsum] @ [WoS ; bv_h @ WoS]   (WoS = sum_h Wo[64h:64h+64])

v2 design vs v1:
  * query/key/value are transposed + cast to bf16 on host -> no PE
    transposes on device, half the input DMA bytes.
  * causal mask is multiplicative (1/0 bf16, applied to exp'd scores on
    DVE) instead of additive -1e9 pre-exp.
  * v projected directly into [row, hd] layout (stationary xT operand).
  * output is written unnormalized in bf16 together with the per-row
    expsum; the softmax division happens on host.

Sharding: 8 cores = 4 batches x 2 balanced causal halves of the query rows
(half 0: rows 0:512 and 1536:2048; half 1: rows 512:1536). Each core
redundantly computes kh/vh for its batch. Uniform SPMD program.
"""

import sys

sys.path.insert(0, "/opt/trn_rl_repo")

from contextlib import ExitStack

import numpy as np

import concourse.bass as bass
import concourse.tile as tile
from concourse import bacc
from concourse import mybir
from concourse.bass_utils import run_bass_kernel_spmd

B, S, D, H, HD = 4, 2048, 1024, 16, 64
P = 128
NCORES = 8

F32 = mybir.dt.float32
F32R = mybir.dt.float32r
BF16 = mybir.dt.bfloat16

# window setup: every core has 2 q-windows of 512 rows.
#   core c: b = c//2, h = c%2
#   Q0(w=0) = 512*h          (slots: 8)
#   Q0(w=1) = 1536 - 512*h   (slots: 16)
W_SLOTS = (8, 16)
BAND = 1408  # mask band width: 128*7 + 512

# wp: packed bf16 const [weights wq|wk|wv (512 each) | ones(16)]
WP_Q, WP_K, WP_V, WP_ONES = 0, 512, 1024, 1536
WP_N = 1552


def _q0(h, w):
    return 512 * h if w == 0 else 1536 - 512 * h


def _r(ap, dt):
    return ap.bitcast(dt)


def _emit(tc, io):
    """Emit the per-core program. io: dict of DRAM APs."""
    nc = tc.nc
    ctx = ExitStack()
    with ctx:
        # ---------------- pools ----------------
        const = ctx.enter_context(tc.tile_pool(name="const", bufs=1))
        atp = ctx.enter_context(tc.tile_pool(name="attn", bufs=8))
        ohp = ctx.enter_context(tc.tile_pool(name="oh", bufs=2))
        opool = ctx.enter_context(tc.tile_pool(name="o", bufs=4))

        ps_s = ctx.enter_context(tc.tile_pool(name="ps_s", bufs=2, space="PSUM"))
        ps_p = ctx.enter_context(tc.tile_pool(name="ps_p", bufs=2, space="PSUM"))
        ps_o = ctx.enter_context(tc.tile_pool(name="ps_o", bufs=2, space="PSUM"))

        # ---------------- resident SBUF tiles ----------------
        wp = const.tile([P, WP_N], BF16, tag="wp")
        wos = const.tile([65, D + 2], F32R, tag="wos")
        mu = const.tile([P, 2 * BAND], BF16, tag="mu")  # generated on device
        mg = const.tile([2, 128 + 2 * BAND], F32R, tag="mg")
        xq = const.tile([P, 8 * 1024], BF16, tag="xq")  # [p, (j, r)] j-chunk major
        # k and v share one tile: chunks 0..7 = k^T, 8..15 = v^T
        xkv = const.tile([P, 16 * S], BF16, tag="xkv")
        qh = const.tile([HD, 1024], F32R, tag="qh")
        kh = const.tile([HD, S], F32R, tag="kh")
        vh = const.tile([P, 16 * 65], BF16, tag="vh")  # [:, 65c:65c+64] + ones col

        # ---------------- input DMAs (front-loaded, need-order) ----------
        # Issue alternates between the SP (HWDGE) and Pool (SWDGE) queues:
        # each queue's per-DMA setup overhead overlaps the other queue's
        # transfer, keeping the DMA engines saturated.
        def dma_kv(eng, c0, nb):
            # one DMA covers both k and v chunks of the group: arrival order
            # within a group (k before v) is guaranteed by the single
            # transfer
            eng.dma_start(
                xkv[:].rearrange("p (j r) -> p j r", r=S)[:, :, 128 * c0 : 128 * (c0 + nb)],
                io["xkv"].rearrange("(j p) r -> p j r", p=P)[:, :, 128 * c0 : 128 * (c0 + nb)],
            )

        # ones column at vh[:, 65c+64] (no DMA: Pool memset)
        nc.gpsimd.memset(vh[:].rearrange("p (c e) -> p c e", e=65)[:, :, 64:65], 1.0)
        # mgen + wos ride the Act queue (idle until the first projection
        # evac); the big transfers alternate SP/Pool so each queue's per-DMA
        # overhead hides under the other queue's transfer.
        nc.scalar.dma_start(mg[:], _r(io["mgen"][:], F32R))
        nc.scalar.dma_start(wos[:], _r(io["wos"][:], F32R))
        nc.sync.dma_start(wp[:], io["wp"][:])
        nc.sync.dma_start(
            xq[:].rearrange("p (j r) -> p j r", r=1024),
            io["xqT"].rearrange("(j p) r -> p j r", p=P),
        )
        GROUPS = [(0, 4), (4, 4), (8, 4), (12, 2), (14, 2)]
        for c0, nb in GROUPS:
            dma_kv(nc.sync, c0, nb)

        # ---------------- device-side causal masks ----------------------
        # mu[p, w*BAND + u] = 1.0 iff p <= u - X_w, via a rank-2 PE outer
        # product (iota difference) and a DVE >=0 compare. Runs in the
        # DMA-bound opening microseconds and warms up the PE p-state.
        for w in range(2):
            for c0, cw in ((0, 512), (512, 512), (1024, BAND - 1024)):
                psm = ps_s.tile([P, 1024], F32, tag="ps")
                # redundant rewrites: keeps the PE p-state ramp warm through
                # the DMA-bound opening (the value is identical each time)
                for _ in range(2):
                    nc.tensor.matmul(
                        psm[:, 0:cw],
                        mg[:, 0:128],
                        mg[:, 128 + BAND * w + c0 : 128 + BAND * w + c0 + cw],
                        start=True,
                        stop=True,
                    )
                nc.vector.tensor_scalar(
                    mu[:, BAND * w + c0 : BAND * w + c0 + cw],
                    psm[:, 0:cw],
                    0.0,
                    None,
                    mybir.AluOpType.is_ge,
                )

        # ---------------- projections ----------------
        def proj_qk(dst, w_off, x_sb, xoff, xstride, bias):
            psp_full = ps_p.tile([P, 512], F32, tag="pp")
            psp = psp_full[0:HD]
            for j in range(8):
                nc.tensor.matmul(
                    psp,
                    wp[:, w_off + 64 * j : w_off + 64 * j + 64],
                    x_sb[:, xoff + xstride * j : xoff + xstride * j + 512],
                    start=(j == 0),
                    stop=(j == 7),
                )
            nc.scalar.activation(
                dst, psp, mybir.ActivationFunctionType.Identity, bias=bias, scale=1.0
            )

        def proj_q(w):
            proj_qk(
                qh[:, 512 * w : 512 * w + 512], WP_Q, xq, 512 * w, 1024,
                _r(wos[0:HD, D : D + 1], F32)
            )

        def proj_k(c0, nb):
            w_cols = 128 * nb
            psp_full = ps_p.tile([P, 512], F32, tag="pp")
            psp = psp_full[0:HD][:, 0:w_cols]
            for j in range(8):
                nc.tensor.matmul(
                    psp,
                    wp[:, WP_K + 64 * j : WP_K + 64 * j + 64],
                    xkv[:, S * j + 128 * c0 : S * j + 128 * c0 + w_cols],
                    start=(j == 0),
                    stop=(j == 7),
                )
            # evac on DVE (tensor_scalar add of the per-partition bias) so
            # the Act queue stays a pure exp stream during slot phases
            nc.vector.tensor_scalar_add(
                kh[:, 128 * c0 : 128 * c0 + w_cols], psp,
                _r(wos[0:HD, D + 1 : D + 2], F32),
            )

        def proj_v(c0, nb):
            # orientation B: out[row, hd]; stationary xT block, moving weight
            for t in range(nb):
                c = c0 + t
                psv = ps_p.tile([P, 512], F32, tag="pp")
                for j in range(8):
                    nc.tensor.matmul(
                        psv[:, 0:HD],
                        xkv[:, S * (8 + j) + 128 * c : S * (8 + j) + 128 * c + 128],
                        wp[:, WP_V + 64 * j : WP_V + 64 * j + 64],
                        start=(j == 0),
                        stop=(j == 7),
                    )
                nc.vector.tensor_copy(vh[:, 65 * c : 65 * c + HD], psv[:, 0:HD])

        # ---------------- attention ----------------
        pso0 = ps_o.tile([65, 512], F32, tag="po")
        pso1 = ps_o.tile([65, 512], F32, tag="po")
        psos = [pso0, pso1]

        def attn_group(c0, nb):
            """Scores + exp (+ causal mask) for k-blocks c0..c0+nb-1.

            W1 and (for g<2) W0 share one [P, 1024] PSUM tile and a single
            exp: halves the Act instruction count in the hot slot phase.
            Returns the AV operands in accumulation order.
            """
            pend = []
            for j in range(c0, c0 + nb):
                paired = j < 8
                width = 1024 if paired else 512
                pss = ps_s.tile([P, 1024], F32, tag="ps")
                khj = kh[:, 128 * j : 128 * j + 128]
                nc.tensor.matmul(
                    pss[:, 0:512], khj, qh[:, 512:1024], start=True, stop=True
                )
                if paired:
                    nc.tensor.matmul(
                        pss[:, 512:1024], khj, qh[:, 0:512], start=True, stop=True
                    )
                at = atp.tile([P, 1024], BF16, tag="at")
                nc.scalar.activation(
                    at[:, 0:width], pss[:, 0:width],
                    mybir.ActivationFunctionType.Exp,
                )
                need0 = paired  # W0 is always on the causal boundary
                need1 = j >= 8
                am = None
                if need0 or need1:
                    am = atp.tile([P, 1024], BF16, tag="am")
                if need1:
                    toff = P * (15 - j)
                    nc.vector.tensor_mul(
                        am[:, 0:512], at[:, 0:512],
                        mu[:, BAND + toff : BAND + toff + 512],
                    )
                    pend.append((1, j, am[:, 0:512]))
                else:
                    pend.append((1, j, at[:, 0:512]))
                if paired:
                    toff = P * (7 - j)
                    nc.vector.tensor_mul(
                        am[:, 512:1024], at[:, 512:1024],
                        mu[:, toff : toff + 512],
                    )
                    pend.append((0, j, am[:, 512:1024]))
            return pend

        def slot_av(w, j, src):
            nc.tensor.matmul(
                psos[w][:],
                vh[:, 65 * j : 65 * j + 65],
                src,
                start=(j == 0),
                stop=(j == W_SLOTS[w] - 1),
            )

        def finish_window(w):
            oh = ohp.tile([65, 512], F32R, tag="oh")
            nc.vector.tensor_copy(oh[:], psos[w][:])
            nc.sync.dma_start(io["esum"][w : w + 1, :], _r(oh[64:65, :], F32))
            for t in range(4):
                ot = opool.tile([P, D], BF16, tag="o")
                for nh in range(2):
                    if (t + nh) % 2 == 0:
                        psf_full = ps_s.tile([P, 1024], F32, tag="ps")
                        psf = psf_full[:, 0:512]
                    else:
                        psf = ps_p.tile([P, 512], F32, tag="pp")
                    nc.tensor.matmul(
                        psf,
                        oh[:, 128 * t : 128 * t + 128],
                        wos[:, 512 * nh : 512 * nh + 512],
                        start=True,
                        stop=True,
                    )
                    dst = ot[:, 512 * nh : 512 * nh + 512]
                    if (t + nh) % 2 == 0:
                        nc.vector.tensor_copy(dst, psf)
                    else:
                        nc.scalar.copy(dst, psf)
                # final window drains on SP (streams back-to-back, cheap
                # HWDGE gen); W0 outs ride Pool mid-stream
                eng = nc.sync if w == 1 else nc.gpsimd
                eng.dma_start(
                    io["out"][512 * w + 128 * t : 512 * w + 128 * t + 128, :], ot[:]
                )

        # ---------------- schedule ----------------
        proj_q(0)
        proj_q(1)
        proj_k(*GROUPS[0])
        for gi, (c0, nb) in enumerate(GROUPS[:-2]):
            pend = attn_group(c0, nb)
            proj_v(c0, nb)
            half = len(pend) // 2
            for w2, j2, s2 in pend[:half]:
                slot_av(w2, j2, s2)
            proj_k(*GROUPS[gi + 1])
            for w2, j2, s2 in pend[half:]:
                slot_av(w2, j2, s2)
            if c0 + nb == 8:
                finish_window(0)
        # latency-bound tail: emit both fine groups' scores/exps before any
        # of their AVs so the two 2-block groups overlap fully
        pend = attn_group(*GROUPS[-2])
        proj_v(*GROUPS[-2])
        proj_k(*GROUPS[-1])
        pend += attn_group(*GROUPS[-1])
        proj_v(*GROUPS[-1])
        for w2, j2, s2 in pend:
            slot_av(w2, j2, s2)
        finish_window(1)


_CACHE = {}


def _build():
    if "nc" in _CACHE:
        return _CACHE["nc"]
    nc = bacc.Bacc("TRN2", target_bir_lowering=False, debug=False, num_devices=NCORES)
    io = {}
    io["xqT"] = nc.dram_tensor("xqT", [D, 1024], BF16, kind="ExternalInput").ap()
    io["xkv"] = nc.dram_tensor("xkv", [2 * D, S], BF16, kind="ExternalInput").ap()
    io["wp"] = nc.dram_tensor("wp", [P, WP_N], BF16, kind="ExternalInput").ap()
    io["wos"] = nc.dram_tensor("wos", [65, D + 2], F32, kind="ExternalInput").ap()
    io["mgen"] = nc.dram_tensor("mgen", [2, 128 + 2 * BAND], F32, kind="ExternalInput").ap()
    io["out"] = nc.dram_tensor("out", [1024, D], BF16, kind="ExternalOutput").ap()
    io["esum"] = nc.dram_tensor("esum", [2, 512], F32, kind="ExternalOutput").ap()
    with tile.TileContext(nc) as tc:
        _emit(tc, io)
    nc.compile()
    _CACHE["nc"] = nc
    return nc


def _host_prep(query, key, value, mask, Wq, Wk, Wv, wq_h, bq_h, wk_h, bk_h, wv_h,
               bv_h, Wo):
    """Combine weights on host (exact algebra, float64 accumulate)."""
    Aq = (np.asarray(Wq, np.float64) @ np.asarray(wq_h, np.float64) / 8.0).astype(
        np.float32
    )
    Ak = (np.asarray(Wk, np.float64) @ np.asarray(wk_h, np.float64)).astype(np.float32)
    Av = (np.asarray(Wv, np.float64) @ np.asarray(wv_h, np.float64)).astype(np.float32)
    bq = (np.asarray(bq_h, np.float64) / 8.0).astype(np.float32)
    bk = np.asarray(bk_h, np.float32)
    WoS = np.asarray(Wo, np.float64).reshape(H, HD, D).sum(axis=0)
    wos_aug = np.concatenate(
        [WoS, (np.asarray(bv_h, np.float64) @ WoS)[None, :]], axis=0
    ).astype(np.float32)
    # biases ride in two extra columns: col D = bq (rows 0:64), col D+1 = bk
    wos_ext = np.zeros((65, D + 2), np.float32)
    wos_ext[:, 0:D] = wos_aug
    wos_ext[0:HD, D] = bq
    wos_ext[0:HD, D + 1] = bk
    return Aq, Ak, Av, wos_ext


def _pack_w(A):
    """[1024, 64] -> [128, 512] partition-packed layout."""
    return np.ascontiguousarray(
        A.reshape(8, P, HD).transpose(1, 0, 2).reshape(P, 512)
    )


def _mk_mgen(h):
    """Inputs for device-side mask generation.

    Row 0: [ones(128) | u - X_0 | u - X_1]; row 1: [iota(128) | -1 | -1].
    The PE computes psm[p, u] = (u - X_w) - p; keep iff >= 0.
    """
    u = np.arange(BAND, dtype=np.float32)
    mgen = np.empty((2, 128 + 2 * BAND), np.float32)
    mgen[0, 0:128] = 1.0
    mgen[1, 0:128] = np.arange(128, dtype=np.float32)
    mgen[1, 128:] = -1.0
    mgen[0, 128 : 128 + BAND] = u - np.float32(896 - _q0(h, 0))
    mgen[0, 128 + BAND :] = u - np.float32(1920 - _q0(h, 1))
    return mgen


def _numpy_fallback(query, key, value, mask, Wq, Wk, Wv, wq_h, bq_h, wk_h, bk_h,
                    wv_h, bv_h, Wo):
    q = query @ Wq
    k = key @ Wk
    v = value @ Wv
    qh = q @ wq_h + bq_h
    kh = k @ wk_h + bk_h
    vh = v @ wv_h + bv_h
    scores = np.einsum("bsh,bth->bst", qh, kh) / np.sqrt(np.float32(HD))
    scores = np.where(mask, np.float32(-1e9), scores)
    scores = scores - scores.max(axis=-1, keepdims=True)
    e = np.exp(scores)
    attn = e / e.sum(axis=-1, keepdims=True)
    out_h = np.einsum("bst,bth->bsh", attn, vh)
    out = np.tile(out_h, (1, 1, H))
    return (out @ Wo).astype(np.float32)


def kernel(**inputs):
    import ml_dtypes

    inputs = {k: np.asarray(v) for k, v in inputs.items()}
    mask = inputs["mask"]
    causal = np.array_equal(mask, np.triu(np.ones((S, S), bool), k=1))
    if not causal:
        return _numpy_fallback(**inputs)

    query, key, value = inputs["query"], inputs["key"], inputs["value"]
    Aq, Ak, Av, wos_ext = _host_prep(**inputs)

    wp = np.zeros((P, WP_N), ml_dtypes.bfloat16)
    wp[:, WP_Q : WP_Q + 512] = _pack_w(Aq).astype(ml_dtypes.bfloat16)
    wp[:, WP_K : WP_K + 512] = _pack_w(Ak).astype(ml_dtypes.bfloat16)
    wp[:, WP_V : WP_V + 512] = _pack_w(Av).astype(ml_dtypes.bfloat16)
    wp[:, WP_ONES : WP_ONES + 16] = np.float32(1.0)

    nc = _build()
    xkv = {}
    for b in range(B):
        buf = np.empty((2 * D, S), ml_dtypes.bfloat16)
        buf[0:D] = key[b].T.astype(ml_dtypes.bfloat16)
        buf[D:] = value[b].T.astype(ml_dtypes.bfloat16)
        xkv[b] = buf
    in_maps = []
    for c in range(NCORES):
        b, h = c // 2, c % 2
        xq_rows = np.concatenate(
            [
                query[b, _q0(h, 0) : _q0(h, 0) + 512],
                query[b, _q0(h, 1) : _q0(h, 1) + 512],
            ],
            axis=0,
        )
        in_maps.append(
            {
                "xqT": np.ascontiguousarray(xq_rows.T.astype(ml_dtypes.bfloat16)),
                "xkv": xkv[b],
                "wp": wp,
                "wos": wos_ext,
                "mgen": _mk_mgen(h),
            }
        )

    res = run_bass_kernel_spmd(nc, in_maps, list(range(NCORES)))
    out = np.empty((B, S, D), np.float32)
    for c in range(NCORES):
        b, h = c // 2, c % 2
        co = np.asarray(res.results[c]["out"]).astype(np.float32)
        es = np.asarray(res.results[c]["esum"]).astype(np.float32)
        co[0:512] /= es[0][:, None]
        co[512:1024] /= es[1][:, None]
        out[b, _q0(h, 0) : _q0(h, 0) + 512] = co[0:512]
        out[b, _q0(h, 1) : _q0(h, 1) + 512] = co[512:1024]
    return out


if __name__ == "__main__":
    nc = _build()
    print("build ok")


# revision 6
# speedup vs baseline: 1.0466x; 1.0466x over previous
"""Trainium2 Bass kernel for nn_MultiHeadAttention_56676388438432 (v2).

Reference math (all H=16 heads share identical weights, so they collapse):
    q = query @ Wq; k = key @ Wk; v = value @ Wv          (full-width, [B,S,D])
    qh = q @ wq_h + bq_h                                   ([B,S,64])
    scores = qh @ kh^T / 8, causal mask, softmax
    out_h = attn @ vh
    out = tile(out_h, 16) @ Wo

Algebraic collapse (exact):
    qh = query @ (Wq @ wq_h / 8) + bq_h/8   -> combined Aq [1024, 64]
    kh = key   @ (Wk @ wk_h) + bk_h         -> combined Ak
    vh = value @ (Wv @ wv_h)        (bv_h folded into Wo, see below)
    out = [out_h ; expsum] @ [WoS ; bv_h @ WoS]   (WoS = sum_h Wo[64h:64h+64])

v2 design vs v1:
  * query/key/value are transposed + cast to bf16 on host -> no PE
    transposes on device, half the input DMA bytes.
  * causal mask is multiplicative (1/0 bf16, applied to exp'd scores on
    DVE) instead of additive -1e9 pre-exp.
  * v projected directly into [row, hd] layout (stationary xT operand).
  * output is written unnormalized in bf16 together with the per-row
    expsum; the softmax division happens on host.

Sharding: 8 cores = 4 batches x 2 balanced causal halves of the query rows
(half 0: rows 0:512 and 1536:2048; half 1: rows 512:1536). Each core
redundantly computes kh/vh for its batch. Uniform SPMD program.
"""

import sys

sys.path.insert(0, "/opt/trn_rl_repo")

from contextlib import ExitStack

import numpy as np

import concourse.bass as bass
import concourse.tile as tile
from concourse import bacc
from concourse import mybir
from concourse.bass_utils import run_bass_kernel_spmd

B, S, D, H, HD = 4, 2048, 1024, 16, 64
P = 128
NCORES = 8

F32 = mybir.dt.float32
F32R = mybir.dt.float32r
BF16 = mybir.dt.bfloat16

# window setup: every core has 2 q-windows of 512 rows.
#   core c: b = c//2, h = c%2
#   Q0(w=0) = 512*h          (slots: 8)
#   Q0(w=1) = 1536 - 512*h   (slots: 16)
W_SLOTS = (8, 16)
BAND = 1408  # mask band width: 128*7 + 512

# wp: packed bf16 const [weights wq|wk|wv (512 each) | ones(16)]
WP_Q, WP_K, WP_V, WP_ONES = 0, 512, 1024, 1536
WP_N = 1552


def _q0(h, w):
    return 512 * h if w == 0 else 1536 - 512 * h


def _r(ap, dt):
    return ap.bitcast(dt)


def _emit(tc, io):
    """Emit the per-core program. io: dict of DRAM APs."""
    nc = tc.nc
    ctx = ExitStack()
    with ctx:
        # ---------------- pools ----------------
        const = ctx.enter_context(tc.tile_pool(name="const", bufs=1))
        atp = ctx.enter_context(tc.tile_pool(name="attn", bufs=8))
        ohp = ctx.enter_context(tc.tile_pool(name="oh", bufs=2))
        opool = ctx.enter_context(tc.tile_pool(name="o", bufs=4))

        ps_s = ctx.enter_context(tc.tile_pool(name="ps_s", bufs=2, space="PSUM"))
        ps_p = ctx.enter_context(tc.tile_pool(name="ps_p", bufs=2, space="PSUM"))
        ps_o = ctx.enter_context(tc.tile_pool(name="ps_o", bufs=2, space="PSUM"))

        # ---------------- resident SBUF tiles ----------------
        wp = const.tile([P, WP_N], BF16, tag="wp")
        wos = const.tile([65, D + 2], F32R, tag="wos")
        mu = const.tile([P, 2 * BAND], BF16, tag="mu")  # generated on device
        mg = const.tile([2, 128 + 2 * BAND], F32R, tag="mg")
        xq = const.tile([P, 8 * 1024], BF16, tag="xq")  # [p, (j, r)] j-chunk major
        # k and v share one tile: chunks 0..7 = k^T, 8..15 = v^T
        xkv = const.tile([P, 16 * S], BF16, tag="xkv")
        qh = const.tile([HD, 1024], F32R, tag="qh")
        kh = const.tile([HD, S], F32R, tag="kh")
        vh = const.tile([P, 16 * 65], BF16, tag="vh")  # [:, 65c:65c+64] + ones col

        # ---------------- input DMAs (front-loaded, need-order) ----------
        # Issue alternates between the SP (HWDGE) and Pool (SWDGE) queues:
        # each queue's per-DMA setup overhead overlaps the other queue's
        # transfer, keeping the DMA engines saturated.
        def dma_kv(eng, c0, nb):
            # one DMA covers both k and v chunks of the group: arrival order
            # within a group (k before v) is guaranteed by the single
            # transfer
            eng.dma_start(
                xkv[:].rearrange("p (j r) -> p j r", r=S)[:, :, 128 * c0 : 128 * (c0 + nb)],
                io["xkv"].rearrange("(j p) r -> p j r", p=P)[:, :, 128 * c0 : 128 * (c0 + nb)],
            )

        # ones column at vh[:, 65c+64] (no DMA: Pool memset)
        nc.gpsimd.memset(vh[:].rearrange("p (c e) -> p c e", e=65)[:, :, 64:65], 1.0)
        # mgen + wos ride the Act queue (idle until the first projection
        # evac); the big transfers alternate SP/Pool so each queue's per-DMA
        # overhead hides under the other queue's transfer.
        nc.scalar.dma_start(mg[:], _r(io["mgen"][:], F32R))
        nc.scalar.dma_start(wos[:], _r(io["wos"][:], F32R))
        nc.sync.dma_start(wp[:], io["wp"][:])
        nc.sync.dma_start(
            xq[:].rearrange("p (j r) -> p j r", r=1024),
            io["xqT"].rearrange("(j p) r -> p j r", p=P),
        )
        GROUPS = [(0, 2), (2, 2), (4, 2), (6, 2), (8, 3), (11, 3), (14, 2)]
        for c0, nb in GROUPS:
            dma_kv(nc.sync, c0, nb)

        # ---------------- device-side causal masks ----------------------
        # mu[p, w*BAND + u] = 1.0 iff p <= u - X_w, via a rank-2 PE outer
        # product (iota difference) and a DVE >=0 compare. Runs in the
        # DMA-bound opening microseconds and warms up the PE p-state.
        for w in range(2):
            for c0, cw in ((0, 512), (512, 512), (1024, BAND - 1024)):
                psm = ps_s.tile([P, 1024], F32, tag="ps")
                # redundant rewrites: keeps the PE p-state ramp warm through
                # the DMA-bound opening (the value is identical each time)
                for _ in range(2):
                    nc.tensor.matmul(
                        psm[:, 0:cw],
                        mg[:, 0:128],
                        mg[:, 128 + BAND * w + c0 : 128 + BAND * w + c0 + cw],
                        start=True,
                        stop=True,
                    )
                nc.vector.tensor_scalar(
                    mu[:, BAND * w + c0 : BAND * w + c0 + cw],
                    psm[:, 0:cw],
                    0.0,
                    None,
                    mybir.AluOpType.is_ge,
                )

        # ---------------- projections ----------------
        def proj_qk(dst, w_off, x_sb, xoff, xstride, bias):
            psp_full = ps_p.tile([P, 512], F32, tag="pp")
            psp = psp_full[0:HD]
            for j in range(8):
                nc.tensor.matmul(
                    psp,
                    wp[:, w_off + 64 * j : w_off + 64 * j + 64],
                    x_sb[:, xoff + xstride * j : xoff + xstride * j + 512],
                    start=(j == 0),
                    stop=(j == 7),
                )
            nc.scalar.activation(
                dst, psp, mybir.ActivationFunctionType.Identity, bias=bias, scale=1.0
            )

        def proj_q(w):
            proj_qk(
                qh[:, 512 * w : 512 * w + 512], WP_Q, xq, 512 * w, 1024,
                _r(wos[0:HD, D : D + 1], F32)
            )

        def proj_k(c0, nb):
            w_cols = 128 * nb
            psp_full = ps_p.tile([P, 512], F32, tag="pp")
            psp = psp_full[0:HD][:, 0:w_cols]
            for j in range(8):
                nc.tensor.matmul(
                    psp,
                    wp[:, WP_K + 64 * j : WP_K + 64 * j + 64],
                    xkv[:, S * j + 128 * c0 : S * j + 128 * c0 + w_cols],
                    start=(j == 0),
                    stop=(j == 7),
                )
            # evac on DVE (tensor_scalar add of the per-partition bias) so
            # the Act queue stays a pure exp stream during slot phases
            nc.vector.tensor_scalar_add(
                kh[:, 128 * c0 : 128 * c0 + w_cols], psp,
                _r(wos[0:HD, D + 1 : D + 2], F32),
            )

        def proj_v(c0, nb):
            # orientation B: out[row, hd]; stationary xT block, moving weight
            for t in range(nb):
                c = c0 + t
                psv = ps_p.tile([P, 512], F32, tag="pp")
                for j in range(8):
                    nc.tensor.matmul(
                        psv[:, 0:HD],
                        xkv[:, S * (8 + j) + 128 * c : S * (8 + j) + 128 * c + 128],
                        wp[:, WP_V + 64 * j : WP_V + 64 * j + 64],
                        start=(j == 0),
                        stop=(j == 7),
                    )
                nc.vector.tensor_copy(vh[:, 65 * c : 65 * c + HD], psv[:, 0:HD])

        # ---------------- attention ----------------
        pso0 = ps_o.tile([65, 512], F32, tag="po")
        pso1 = ps_o.tile([65, 512], F32, tag="po")
        psos = [pso0, pso1]

        def attn_group(c0, nb):
            """Scores + exp (+ causal mask) for k-blocks c0..c0+nb-1.

            W1 and (for g<2) W0 share one [P, 1024] PSUM tile and a single
            exp: halves the Act instruction count in the hot slot phase.
            Returns the AV operands in accumulation order.
            """
            pend = []
            for j in range(c0, c0 + nb):
                paired = j < 8
                width = 1024 if paired else 512
                pss = ps_s.tile([P, 1024], F32, tag="ps")
                khj = kh[:, 128 * j : 128 * j + 128]
                nc.tensor.matmul(
                    pss[:, 0:512], khj, qh[:, 512:1024], start=True, stop=True
                )
                if paired:
                    nc.tensor.matmul(
                        pss[:, 512:1024], khj, qh[:, 0:512], start=True, stop=True
                    )
                at = atp.tile([P, 1024], BF16, tag="at")
                nc.scalar.activation(
                    at[:, 0:width], pss[:, 0:width],
                    mybir.ActivationFunctionType.Exp,
                )
                need0 = paired  # W0 is always on the causal boundary
                need1 = j >= 8
                am = None
                if need0 or need1:
                    am = atp.tile([P, 1024], BF16, tag="am")
                if need1:
                    toff = P * (15 - j)
                    nc.vector.tensor_mul(
                        am[:, 0:512], at[:, 0:512],
                        mu[:, BAND + toff : BAND + toff + 512],
                    )
                    pend.append((1, j, am[:, 0:512]))
                else:
                    pend.append((1, j, at[:, 0:512]))
                if paired:
                    toff = P * (7 - j)
                    nc.vector.tensor_mul(
                        am[:, 512:1024], at[:, 512:1024],
                        mu[:, toff : toff + 512],
                    )
                    pend.append((0, j, am[:, 512:1024]))
            return pend

        def slot_av(w, j, src):
            nc.tensor.matmul(
                psos[w][:],
                vh[:, 65 * j : 65 * j + 65],
                src,
                start=(j == 0),
                stop=(j == W_SLOTS[w] - 1),
            )

        def finish_window(w):
            oh = ohp.tile([65, 512], F32R, tag="oh")
            nc.vector.tensor_copy(oh[:], psos[w][:])
            nc.sync.dma_start(io["esum"][w : w + 1, :], _r(oh[64:65, :], F32))
            for t in range(4):
                ot = opool.tile([P, D], BF16, tag="o")
                for nh in range(2):
                    if (t + nh) % 2 == 0:
                        psf_full = ps_s.tile([P, 1024], F32, tag="ps")
                        psf = psf_full[:, 0:512]
                    else:
                        psf = ps_p.tile([P, 512], F32, tag="pp")
                    nc.tensor.matmul(
                        psf,
                        oh[:, 128 * t : 128 * t + 128],
                        wos[:, 512 * nh : 512 * nh + 512],
                        start=True,
                        stop=True,
                    )
                    dst = ot[:, 512 * nh : 512 * nh + 512]
                    if (t + nh) % 2 == 0:
                        nc.vector.tensor_copy(dst, psf)
                    else:
                        nc.scalar.copy(dst, psf)
                # final window drains on SP (streams back-to-back, cheap
                # HWDGE gen); W0 outs ride Pool mid-stream
                eng = nc.sync if w == 1 else nc.gpsimd
                eng.dma_start(
                    io["out"][512 * w + 128 * t : 512 * w + 128 * t + 128, :], ot[:]
                )

        # ---------------- schedule ----------------
        proj_q(0)
        proj_q(1)
        proj_k(*GROUPS[0])
        for gi, (c0, nb) in enumerate(GROUPS[:-2]):
            pend = attn_group(c0, nb)
            proj_v(c0, nb)
            half = len(pend) // 2
            for w2, j2, s2 in pend[:half]:
                slot_av(w2, j2, s2)
            proj_k(*GROUPS[gi + 1])
            for w2, j2, s2 in pend[half:]:
                slot_av(w2, j2, s2)
            if c0 < 8 <= c0 + nb:
                finish_window(0)
        # latency-bound tail: emit both fine groups' scores/exps before any
        # of their AVs so the two 2-block groups overlap fully
        pend = attn_group(*GROUPS[-2])
        proj_v(*GROUPS[-2])
        proj_k(*GROUPS[-1])
        pend += attn_group(*GROUPS[-1])
        proj_v(*GROUPS[-1])
        for w2, j2, s2 in pend:
            slot_av(w2, j2, s2)
        finish_window(1)


_CACHE = {}


def _build():
    if "nc" in _CACHE:
        return _CACHE["nc"]
    nc = bacc.Bacc("TRN2", target_bir_lowering=False, debug=False, num_devices=NCORES)
    io = {}
    io["xqT"] = nc.dram_tensor("xqT", [D, 1024], BF16, kind="ExternalInput").ap()
    io["xkv"] = nc.dram_tensor("xkv", [2 * D, S], BF16, kind="ExternalInput").ap()
    io["wp"] = nc.dram_tensor("wp", [P, WP_N], BF16, kind="ExternalInput").ap()
    io["wos"] = nc.dram_tensor("wos", [65, D + 2], F32, kind="ExternalInput").ap()
    io["mgen"] = nc.dram_tensor("mgen", [2, 128 + 2 * BAND], F32, kind="ExternalInput").ap()
    io["out"] = nc.dram_tensor("out", [1024, D], BF16, kind="ExternalOutput").ap()
    io["esum"] = nc.dram_tensor("esum", [2, 512], F32, kind="ExternalOutput").ap()
    with tile.TileContext(nc) as tc:
        _emit(tc, io)
    nc.compile()
    _CACHE["nc"] = nc
    return nc


def _host_prep(query, key, value, mask, Wq, Wk, Wv, wq_h, bq_h, wk_h, bk_h, wv_h,
               bv_h, Wo):
    """Combine weights on host (exact algebra, float64 accumulate)."""
    Aq = (np.asarray(Wq, np.float64) @ np.asarray(wq_h, np.float64) / 8.0).astype(
        np.float32
    )
    Ak = (np.asarray(Wk, np.float64) @ np.asarray(wk_h, np.float64)).astype(np.float32)
    Av = (np.asarray(Wv, np.float64) @ np.asarray(wv_h, np.float64)).astype(np.float32)
    bq = (np.asarray(bq_h, np.float64) / 8.0).astype(np.float32)
    bk = np.asarray(bk_h, np.float32)
    WoS = np.asarray(Wo, np.float64).reshape(H, HD, D).sum(axis=0)
    wos_aug = np.concatenate(
        [WoS, (np.asarray(bv_h, np.float64) @ WoS)[None, :]], axis=0
    ).astype(np.float32)
    # biases ride in two extra columns: col D = bq (rows 0:64), col D+1 = bk
    wos_ext = np.zeros((65, D + 2), np.float32)
    wos_ext[:, 0:D] = wos_aug
    wos_ext[0:HD, D] = bq
    wos_ext[0:HD, D + 1] = bk
    return Aq, Ak, Av, wos_ext


def _pack_w(A):
    """[1024, 64] -> [128, 512] partition-packed layout."""
    return np.ascontiguousarray(
        A.reshape(8, P, HD).transpose(1, 0, 2).reshape(P, 512)
    )


def _mk_mgen(h):
    """Inputs for device-side mask generation.

    Row 0: [ones(128) | u - X_0 | u - X_1]; row 1: [iota(128) | -1 | -1].
    The PE computes psm[p, u] = (u - X_w) - p; keep iff >= 0.
    """
    u = np.arange(BAND, dtype=np.float32)
    mgen = np.empty((2, 128 + 2 * BAND), np.float32)
    mgen[0, 0:128] = 1.0
    mgen[1, 0:128] = np.arange(128, dtype=np.float32)
    mgen[1, 128:] = -1.0
    mgen[0, 128 : 128 + BAND] = u - np.float32(896 - _q0(h, 0))
    mgen[0, 128 + BAND :] = u - np.float32(1920 - _q0(h, 1))
    return mgen


def _numpy_fallback(query, key, value, mask, Wq, Wk, Wv, wq_h, bq_h, wk_h, bk_h,
                    wv_h, bv_h, Wo):
    q = query @ Wq
    k = key @ Wk
    v = value @ Wv
    qh = q @ wq_h + bq_h
    kh = k @ wk_h + bk_h
    vh = v @ wv_h + bv_h
    scores = np.einsum("bsh,bth->bst", qh, kh) / np.sqrt(np.float32(HD))
    scores = np.where(mask, np.float32(-1e9), scores)
    scores = scores - scores.max(axis=-1, keepdims=True)
    e = np.exp(scores)
    attn = e / e.sum(axis=-1, keepdims=True)
    out_h = np.einsum("bst,bth->bsh", attn, vh)
    out = np.tile(out_h, (1, 1, H))
    return (out @ Wo).astype(np.float32)


def kernel(**inputs):
    import ml_dtypes

    inputs = {k: np.asarray(v) for k, v in inputs.items()}
    mask = inputs["mask"]
    causal = np.array_equal(mask, np.triu(np.ones((S, S), bool), k=1))
    if not causal:
        return _numpy_fallback(**inputs)

    query, key, value = inputs["query"], inputs["key"], inputs["value"]
    Aq, Ak, Av, wos_ext = _host_prep(**inputs)

    wp = np.zeros((P, WP_N), ml_dtypes.bfloat16)
    wp[:, WP_Q : WP_Q + 512] = _pack_w(Aq).astype(ml_dtypes.bfloat16)
    wp[:, WP_K : WP_K + 512] = _pack_w(Ak).astype(ml_dtypes.bfloat16)
    wp[:, WP_V : WP_V + 512] = _pack_w(Av).astype(ml_dtypes.bfloat16)
    wp[:, WP_ONES : WP_ONES + 16] = np.float32(1.0)

    nc = _build()
    xkv = {}
    for b in range(B):
        buf = np.empty((2 * D, S), ml_dtypes.bfloat16)
        buf[0:D] = key[b].T.astype(ml_dtypes.bfloat16)
        buf[D:] = value[b].T.astype(ml_dtypes.bfloat16)
        xkv[b] = buf
    in_maps = []
    for c in range(NCORES):
        b, h = c // 2, c % 2
        xq_rows = np.concatenate(
            [
                query[b, _q0(h, 0) : _q0(h, 0) + 512],
                query[b, _q0(h, 1) : _q0(h, 1) + 512],
            ],
            axis=0,
        )
        in_maps.append(
            {
                "xqT": np.ascontiguousarray(xq_rows.T.astype(ml_dtypes.bfloat16)),
                "xkv": xkv[b],
                "wp": wp,
                "wos": wos_ext,
                "mgen": _mk_mgen(h),
            }
        )

    res = run_bass_kernel_spmd(nc, in_maps, list(range(NCORES)))
    out = np.empty((B, S, D), np.float32)
    for c in range(NCORES):
        b, h = c // 2, c % 2
        co = np.asarray(res.results[c]["out"]).astype(np.float32)
        es = np.asarray(res.results[c]["esum"]).astype(np.float32)
        co[0:512] /= es[0][:, None]
        co[512:1024] /= es[1][:, None]
        out[b, _q0(h, 0) : _q0(h, 0) + 512] = co[0:512]
        out[b, _q0(h, 1) : _q0(h, 1) + 512] = co[512:1024]
    return out


if __name__ == "__main__":
    nc = _build()
    print("build ok")
